# revision 1
# baseline (speedup 1.0000x reference)
"""GAT (2-layer, PyG-style) distributed Bass kernel for 8 Trainium2 NeuronCores.

Strategy (graph/data parallel, per sharding hint):
  - Nodes are partitioned into 8 contiguous blocks; core c owns destination
    nodes [c*N/8, (c+1)*N/8) and all edges incident to them (plus self loops).
  - Layer 1: every core builds the full node feature table
    xh1ext = x @ [W1 | W1@a_src_bd] (redundant compute is cheaper than
    cross-core collectives here), then processes its destination tiles:
    a hardware dma_gather fetches per-edge source rows [xh(128) | e_src(8)],
    attention coefficients are formed with leaky_relu+exp (no max-subtraction
    needed: scores are O(1) so exp never overflows; softmax is exactly
    equivalent), and a 0/1 selection-matrix matmul on the tensor engine
    performs the per-destination segment reduction of [msg | ea] in PSUM.
  - Host reassembles the transposed hidden table h_T from the 8 shards
    (pure data movement), then launch 2 repeats the same structure with
    42-wide features for the single-head output layer.

SPMD constraints force fully uniform static structure across cores: every
(dst-tile x src-quarter) edge segment is padded to S chunks of 128 edges
(pad edges gather row 0 and use an out-of-range dst slot so selection
matrices zero them out). Source indices are split into 4 quarters because
dma_gather indices are int16.
"""

import math
import os
import sys

for _p in ("/opt/trn_rl_repo", "/root/.axon_site/_ro/trn_rl_repo"):
    if os.path.isdir(_p) and _p not in sys.path:
        sys.path.insert(0, _p)

import numpy as np
import ml_dtypes
from contextlib import ExitStack

import concourse.bacc as bacc
import concourse.bass as bass
import concourse.tile as tile
from concourse import mybir
from concourse.bass_utils import run_bass_kernel_spmd

F32 = mybir.dt.float32
BF16 = mybir.dt.bfloat16
I16 = mybir.dt.int16
AF = mybir.ActivationFunctionType
ALU = mybir.AluOpType

NEG_SLOPE = 0.2
PSUM_PP_BUFS = 1
EPS = 1e-16
P = 128
PAD_DST = 200.0  # sentinel dst_local for pad edges; never matches iota 0..127


# --------------------------------------------------------------------------
# host-side graph preprocessing
# --------------------------------------------------------------------------

def _round_up(a, b):
    return (a + b - 1) // b * b


class EdgeStruct:
    """Uniform SPMD edge layout shared by both layers."""

    def __init__(self, src, dst, N, n_cores, G=3):
        self.N = N
        self.n_cores = n_cores
        self.G = G
        self.Npad = _round_up(N, 512)
        self.Qsz = self.Npad // 4
        assert self.Qsz <= 32767
        assert N % n_cores == 0
        self.npc = N // n_cores                      # dst nodes per core
        self.T = math.ceil(self.npc / P)             # real dst tiles per core
        self.T_pad = _round_up(self.T, G)
        self.n_groups = self.T_pad // G
        nseg = self.T_pad * 4

        src = src.astype(np.int64)
        dst = dst.astype(np.int64)

        per_core = []
        max_cnt = 0
        for c in range(n_cores):
            lo = c * self.npc
            sel = (dst >= lo) & (dst < lo + self.npc)
            s_c = src[sel]
            dl = dst[sel] - lo                        # local dst id
            t_all = dl >> 7                           # dst tile
            q_all = s_c // self.Qsz                   # src quarter
            key = t_all * 4 + q_all
            order = np.argsort(key, kind="stable")
            s_c, dl, key = s_c[order], dl[order], key[order]
            cnt = np.bincount(key, minlength=nseg)
            max_cnt = max(max_cnt, int(cnt.max()))
            per_core.append((s_c, dl, key, cnt))

        self.S = max(1, math.ceil(max_cnt / P))      # chunks per segment
        S, G_, Qsz = self.S, G, self.Qsz
        self.ncols = 4 * G * S                       # chunk columns per group
        assert self.ncols <= P, f"ncols={self.ncols} > 128; lower S or G"
        slots_seg = S * P

        self.gidx = []    # [n_groups*4*128, G*S*8] int16
        self.gdl = []     # [n_groups*128, ncols]   bf16
        self.gdr = []     # [n_groups*128, 128]     bf16
        for c in range(n_cores):
            s_c, dl, key, cnt = per_core[c]
            flat_idx = np.zeros(nseg * slots_seg, np.int16)
            flat_dl = np.full(nseg * slots_seg, PAD_DST, np.float32)
            starts = np.concatenate([[0], np.cumsum(cnt)])[:-1]
            # position of each edge inside the padded segment layout
            pos_in_seg = np.arange(len(s_c)) - starts[key]
            base = key * slots_seg
            pos = base + pos_in_seg
            q_of_edge = key % 4
            flat_idx[pos] = (s_c - q_of_edge * Qsz).astype(np.int16)
            flat_dl[pos] = (dl & 127).astype(np.float32)

            # flat layout is segment-major: seg = t*4+q, inside: s*128+p.
            # regroup to gather order: per (g, q): (t_loc, s, p)
            fi = flat_idx.reshape(self.T_pad, 4, S, P)
            fd = flat_dl.reshape(self.T_pad, 4, S, P)
            # -> [n_groups, G, 4, S, P] -> [n_groups, 4, G, S, P]
            fi = fi.reshape(self.n_groups, G_, 4, S, P).transpose(0, 2, 1, 3, 4)
            fd = fd.reshape(self.n_groups, G_, 4, S, P).transpose(0, 2, 1, 3, 4)

            # gather idx arrays: flat i = (t_loc*S+s)*128+p ; wrapped [128, i/16]
            fi2 = fi.reshape(self.n_groups, 4, G_ * S * P)
            w = fi2.reshape(self.n_groups, 4, G_ * S * 8, 16)
            w = np.transpose(w, (0, 1, 3, 2))              # [g, 4, 16, cols16]
            w = np.tile(w, (1, 1, 8, 1))                   # replicate to 128
            self.gidx.append(
                np.ascontiguousarray(w.reshape(self.n_groups * 4 * P, G_ * S * 8))
            )

            # dst_local in both layouts; group buffer col = q*(G*S)+t_loc*S+s
            fcol = fd.reshape(self.n_groups, self.ncols, P)   # [g, c, p]
            gdr = np.zeros((self.n_groups, P, P), np.float32)
            gdr[:, : self.ncols, :] = fcol
            gdl = np.transpose(fcol, (0, 2, 1))               # [g, p, c]
            self.gdl.append(
                np.ascontiguousarray(
                    gdl.reshape(self.n_groups * P, self.ncols)
                ).astype(ml_dtypes.bfloat16)
            )
            self.gdr.append(
                np.ascontiguousarray(gdr.reshape(self.n_groups * P, P)).astype(
                    ml_dtypes.bfloat16
                )
            )


# --------------------------------------------------------------------------
# device kernel builder (shared by both layers)
# --------------------------------------------------------------------------

def build_layer_kernel(es: EdgeStruct, layer: int):
    """layer 1: feat table row [xh1(128)|e_src1(8)|junk], elem 192 f32,
               heads=8, csz=16, epilogue = softmax-div + ELU + transpose out.
       layer 2: row [xh2(40)|e_src2(1)|junk], elem 64 f32, heads=1, csz=40,
               epilogue = softmax-div, row-major out."""
    Npad, T_pad, G, S, ncols = es.Npad, es.T_pad, es.G, es.S, es.ncols
    n_groups, Qsz = es.n_groups, es.Qsz
    if layer == 1:
        ELEM, H, CSZ, WCOLS = 192, 8, 16, 136
    else:
        ELEM, H, CSZ, WCOLS = 64, 1, 40, 41
    # self-loop edges are not in the edge lists; their contribution is added
    # analytically in the tile epilogue from the core's own-node rows.
    MW = H * CSZ                      # message width (128 / 40)
    AW = MW + H                       # [msg | ea] width (136 / 41)

    nc = bacc.Bacc("TRN2", target_bir_lowering=False, debug=False,
                   num_devices=es.n_cores)
    ap = {}
    ap["xT"] = nc.dram_tensor("xT", [P, Npad], F32, kind="ExternalInput").ap()
    ap["xTm"] = nc.dram_tensor("xTm", [P, T_pad * P], F32,
                               kind="ExternalInput").ap()
    ap["wext"] = nc.dram_tensor("wext", [P, WCOLS], F32,
                                kind="ExternalInput").ap()
    ap["brow"] = nc.dram_tensor("brow", [1, WCOLS], F32,
                                kind="ExternalInput").ap()
    ap["ones_f"] = nc.dram_tensor("ones_f", [1, P], F32,
                                  kind="ExternalInput").ap()
    ap["wdst"] = nc.dram_tensor("wdst", [P, H], F32, kind="ExternalInput").ap()
    ap["gidx"] = nc.dram_tensor("gidx", [n_groups * 4 * P, G * S * 8], I16,
                                kind="ExternalInput").ap()
    ap["gdl"] = nc.dram_tensor("gdl", [n_groups * P, ncols], BF16,
                               kind="ExternalInput").ap()
    ap["gdr"] = nc.dram_tensor("gdr", [n_groups * P, P], BF16,
                               kind="ExternalInput").ap()
    ap["iota_bf"] = nc.dram_tensor("iota_bf", [P, P], BF16,
                                   kind="ExternalInput").ap()
    ap["iota_col"] = nc.dram_tensor("iota_col", [P, 1], F32,
                                    kind="ExternalInput").ap()
    ap["ones_bf"] = nc.dram_tensor("ones_bf", [1, P], BF16,
                                   kind="ExternalInput").ap()
    ap["idn"] = nc.dram_tensor("idn", [P, P], F32, kind="ExternalInput").ap()
    if layer == 1:
        out_ap = nc.dram_tensor("hT", [P, T_pad * P], F32,
                                kind="ExternalOutput").ap()
    else:
        out_ap = nc.dram_tensor("logits", [T_pad * P, CSZ], F32,
                                kind="ExternalOutput").ap()
    tbl = nc.dram_tensor("tbl", [Npad, ELEM], F32, kind="Internal").ap()
    own_tbl = nc.dram_tensor("own_tbl", [T_pad * P, WCOLS], F32,
                             kind="Internal").ap()

    with tile.TileContext(nc) as tc, ExitStack() as ctx:
        cpool = ctx.enter_context(tc.tile_pool(name="consts", bufs=1))

        # ---- constants ----
        wext = cpool.tile([P, WCOLS], F32)
        nc.sync.dma_start(wext[:], ap["wext"])
        brow = cpool.tile([1, WCOLS], F32)
        nc.sync.dma_start(brow[:], ap["brow"])
        ones_f = cpool.tile([1, P], F32)
        nc.sync.dma_start(ones_f[:], ap["ones_f"])
        wdst = cpool.tile([P, H], F32)
        nc.sync.dma_start(wdst[:], ap["wdst"])
        iota_bf = cpool.tile([P, P], BF16)
        nc.sync.dma_start(iota_bf[:], ap["iota_bf"])
        iota_col = cpool.tile([P, 1], F32)
        nc.sync.dma_start(iota_col[:], ap["iota_col"])
        ones_bf = cpool.tile([1, P], BF16)
        nc.sync.dma_start(ones_bf[:], ap["ones_bf"])
        idn = cpool.tile([P, P], F32)
        nc.sync.dma_start(idn[:], ap["idn"])
        edst_sb = cpool.tile([P, T_pad * H], F32)

        with tc.tile_pool(name="pre_sb", bufs=4) as psb, \
                tc.tile_pool(name="pre_ps", bufs=2, space="PSUM") as pps:
            # ---- pre-pass A: full feature table ----
            for i in range(Npad // P):
                xt = psb.tile([P, P], F32, tag="xt")
                nc.sync.dma_start(xt[:], ap["xT"][:, i * P:(i + 1) * P])
                ppt = pps.tile([P, WCOLS], F32, tag="ppt")
                nc.tensor.matmul(out=ppt[:], lhsT=xt[:], rhs=wext[:],
                                 start=True, stop=False, skip_group_check=True)
                # bias row: feature-table rows get +bias (attention-score
                # columns of brow are zero); since sum(att)=1 per dst this
                # reproduces "+ bias" after aggregation.
                nc.tensor.matmul(out=ppt[:], lhsT=ones_f[:], rhs=brow[:],
                                 start=False, stop=True, skip_group_check=True)
                ot = psb.tile([P, WCOLS], F32, tag="ot")
                nc.vector.tensor_copy(out=ot[:], in_=ppt[:])
                nc.sync.dma_start(tbl[i * P:(i + 1) * P, 0:WCOLS], ot[:])

            # ---- pre-pass B: own-node rows [xh+b|e_src] (DRAM) and e_dst
            # (SBUF-resident), for e_dst matmuls and self-loop epilogue ----
            for t in range(T_pad):
                xt = psb.tile([P, P], F32, tag="xt2")
                nc.sync.dma_start(xt[:], ap["xTm"][:, t * P:(t + 1) * P])
                po = pps.tile([P, WCOLS], F32, tag="po")
                nc.tensor.matmul(out=po[:], lhsT=xt[:], rhs=wext[:],
                                 start=True, stop=False, skip_group_check=True)
                nc.tensor.matmul(out=po[:], lhsT=ones_f[:], rhs=brow[:],
                                 start=False, stop=True, skip_group_check=True)
                oo = psb.tile([P, WCOLS], F32, tag="oo")
                nc.vector.tensor_copy(out=oo[:], in_=po[:])
                nc.sync.dma_start(own_tbl[t * P:(t + 1) * P, :], oo[:])
                pe = pps.tile([P, H], F32, tag="pe")
                nc.tensor.matmul(out=pe[:], lhsT=xt[:], rhs=wdst[:],
                                 start=True, stop=True)
                nc.vector.tensor_copy(out=edst_sb[:, t * H:(t + 1) * H],
                                      in_=pe[:])

        # ---- edge pass ----
        sb = ctx.enter_context(tc.tile_pool(name="sb", bufs=3))
        gbp = ctx.enter_context(tc.tile_pool(name="gbuf", bufs=2))
        pp = ctx.enter_context(tc.tile_pool(name="pp", bufs=PSUM_PP_BUFS, space="PSUM"))
        ptp = ctx.enter_context(tc.tile_pool(name="ptp", bufs=1,
                                             space="PSUM"))
        pacc = ctx.enter_context(tc.tile_pool(name="pacc", bufs=min(G, 2),
                                              space="PSUM"))
        GSP = G * S * P
        tc.strict_bb_all_engine_barrier()
        for g in range(n_groups):
            if g % 2 == 0:
                tc.strict_bb_all_engine_barrier()
            gb = gbp.tile([P, ncols * ELEM], F32, tag="gb")
            gb3 = gb[:].rearrange("p (c k) -> p c k", k=ELEM)
            idxs = sb.tile([P, 4 * G * S * 8], I16, tag="idx")
            for q in range(4):
                nc.sync.dma_start(
                    idxs[:, q * G * S * 8:(q + 1) * G * S * 8],
                    ap["gidx"][(g * 4 + q) * P:(g * 4 + q + 1) * P, :])
            dlt = sb.tile([P, ncols], BF16, tag="dl")
            nc.sync.dma_start(dlt[:], ap["gdl"][g * P:(g + 1) * P, :])
            drt = sb.tile([P, P], BF16, tag="dr")
            nc.sync.dma_start(drt[:], ap["gdr"][g * P:(g + 1) * P, :])
            MAXC = 4  # sub-gather size in 128-idx chunks (HW-validated regime)
            for q in range(4):
                for c0 in range(0, G * S, MAXC):
                    c1 = min(c0 + MAXC, G * S)
                    nc.gpsimd.dma_gather(
                        out_ap=gb3[:, q * G * S + c0:q * G * S + c1, :],
                        in_ap=tbl[q * Qsz:(q + 1) * Qsz, :],
                        idxs_ap=idxs[:, (q * G * S + c0) * 8:
                                     (q * G * S + c1) * 8],
                        num_idxs=(c1 - c0) * P,
                        num_idxs_reg=(c1 - c0) * P,
                        elem_size=ELEM,
                    )
            for t_loc in range(G):
                t = g * G + t_loc
                acc = pacc.tile([P, AW], F32, tag="acc")
                for q in range(4):
                    for s in range(S):
                        c = q * G * S + t_loc * S + s
                        xh_ch = gb[:, c * ELEM:c * ELEM + MW]
                        es_ch = gb[:, c * ELEM + MW:c * ELEM + MW + H]
                        # S_eT[e,d] = (dst_local[e] == d)
                        seT = sb.tile([P, P], F32, tag="seT")
                        nc.vector.tensor_tensor(
                            out=seT[:],
                            in0=dlt[:, c:c + 1].to_broadcast([P, P]),
                            in1=iota_bf[:], op=ALU.is_equal)
                        # S_dT = transpose(S_eT)
                        bc = pp.tile([P, P], F32, tag="bc")
                        nc.tensor.transpose(out=bc[:], in_=seT[:],
                                            identity=idn[:])
                        sdT = sb.tile([P, P], F32, tag="sdT")
                        nc.vector.tensor_copy(out=sdT[:], in_=bc[:])
                        # e_dst per edge: S_dT.T @ e_dst_tile -> [e, H]
                        ed = pp.tile([P, H], F32, tag="ed")
                        nc.tensor.matmul(
                            out=ed[:], lhsT=sdT[:],
                            rhs=edst_sb[:, t * H:(t + 1) * H],
                            start=True, stop=True)
                        mea = sb.tile([P, AW], F32, tag="mea")
                        al = sb.tile([P, H], F32, tag="al")
                        al2 = sb.tile([P, H], F32, tag="al2")
                        nc.vector.tensor_tensor(out=al[:], in0=es_ch,
                                                in1=ed[:], op=ALU.add)
                        # leaky_relu(a) = max(a, slope*a)
                        nc.vector.tensor_scalar_mul(out=al2[:], in0=al[:],
                                                    scalar1=NEG_SLOPE)
                        nc.vector.tensor_tensor(out=al[:], in0=al[:],
                                                in1=al2[:], op=ALU.max)
                        nc.scalar.activation(out=mea[:, MW:AW], in_=al[:],
                                             func=AF.Exp)
                        # msg = xh * ea (broadcast over channel group)
                        if H == 1:
                            nc.vector.tensor_tensor(
                                out=mea[:, 0:MW],
                                in0=mea[:, MW:AW].to_broadcast([P, MW]),
                                in1=xh_ch, op=ALU.mult)
                        else:
                            ea3 = mea[:, MW:AW].rearrange(
                                "p (h o) -> p h o", o=1).to_broadcast(
                                [P, H, CSZ])
                            xh3 = xh_ch.rearrange("p (h c) -> p h c", c=CSZ)
                            mea3 = mea[:, 0:MW].rearrange(
                                "p (h c) -> p h c", c=CSZ)
                            nc.vector.tensor_tensor(out=mea3, in0=ea3,
                                                    in1=xh3, op=ALU.mult)
                        # segment-reduce into the tile accumulator
                        nc.tensor.matmul(out=acc[:], lhsT=seT[:], rhs=mea[:],
                                         start=(q == 0 and s == 0),
                                         stop=(q == 3 and s == S - 1),
                                         skip_group_check=True)
                # ---- tile epilogue (adds analytic self-loop term) ----
                own = sb.tile([P, WCOLS], F32, tag="own")
                nc.sync.dma_start(own[:], own_tbl[t * P:(t + 1) * P, :])
                als = sb.tile([P, H], F32, tag="als")
                als2 = sb.tile([P, H], F32, tag="als2")
                nc.vector.tensor_tensor(out=als[:], in0=own[:, MW:WCOLS],
                                        in1=edst_sb[:, t * H:(t + 1) * H],
                                        op=ALU.add)
                nc.vector.tensor_scalar_mul(out=als2[:], in0=als[:],
                                            scalar1=NEG_SLOPE)
                nc.vector.tensor_tensor(out=als[:], in0=als[:], in1=als2[:],
                                        op=ALU.max)
                eas = sb.tile([P, H], F32, tag="eas")
                nc.scalar.activation(out=eas[:], in_=als[:], func=AF.Exp)
                # self message: note own xh columns include +bias, matching
                # the gathered table rows.
                smsg = sb.tile([P, MW], F32, tag="smsg")
                if H == 1:
                    nc.vector.tensor_tensor(
                        out=smsg[:], in0=eas[:, 0:1].to_broadcast([P, MW]),
                        in1=own[:, 0:MW], op=ALU.mult)
                else:
                    nc.vector.tensor_tensor(
                        out=smsg[:].rearrange("p (h c) -> p h c", c=CSZ),
                        in0=eas[:].rearrange("p (h o) -> p h o", o=1)
                        .to_broadcast([P, H, CSZ]),
                        in1=own[:, 0:MW].rearrange("p (h c) -> p h c", c=CSZ),
                        op=ALU.mult)
                unorm = sb.tile([P, MW], F32, tag="unorm")
                nc.vector.tensor_tensor(out=unorm[:], in0=acc[:, 0:MW],
                                        in1=smsg[:], op=ALU.add)
                den = sb.tile([P, H], F32, tag="den")
                nc.vector.tensor_tensor(out=den[:], in0=acc[:, MW:AW],
                                        in1=eas[:], op=ALU.add)
                nc.vector.tensor_scalar_add(out=den[:], in0=den[:],
                                            scalar1=EPS)
                rec = sb.tile([P, H], F32, tag="rec")
                nc.vector.reciprocal(out=rec[:], in_=den[:])
                otile = sb.tile([P, MW], F32, tag="otile")
                if H == 1:
                    nc.vector.tensor_tensor(
                        out=otile[:], in0=rec[:, 0:1].to_broadcast([P, MW]),
                        in1=unorm[:], op=ALU.mult)
                else:
                    rec3 = rec[:].rearrange("p (h o) -> p h o", o=1) \
                        .to_broadcast([P, H, CSZ])
                    acc3 = unorm[:].rearrange("p (h c) -> p h c", c=CSZ)
                    ot3 = otile[:].rearrange("p (h c) -> p h c", c=CSZ)
                    nc.vector.tensor_tensor(out=ot3, in0=rec3, in1=acc3,
                                            op=ALU.mult)
                if layer == 1:
                    # ELU then transpose out
                    tmp = sb.tile([P, MW], F32, tag="tmp")
                    nc.vector.tensor_scalar_min(out=tmp[:], in0=otile[:],
                                                scalar1=0.0)
                    nc.scalar.activation(out=tmp[:], in_=tmp[:], func=AF.Exp)
                    nc.scalar.activation(out=otile[:], in_=otile[:],
                                         func=AF.Relu)
                    nc.vector.tensor_tensor(out=otile[:], in0=tmp[:],
                                            in1=otile[:], op=ALU.add)
                    nc.vector.tensor_scalar_add(out=otile[:], in0=otile[:],
                                                scalar1=-1.0)
                    tp = ptp.tile([P, P], F32, tag="tp")
                    nc.tensor.transpose(out=tp[:], in_=otile[:],
                                        identity=idn[:])
                    hTt = sb.tile([P, P], F32, tag="hTt")
                    nc.vector.tensor_copy(out=hTt[:], in_=tp[:])
                    nc.sync.dma_start(out_ap[:, t * P:(t + 1) * P], hTt[:])
                else:
                    nc.sync.dma_start(out_ap[t * P:(t + 1) * P, :], otile[:])

    nc.compile()
    return nc


# --------------------------------------------------------------------------
# host orchestration
# --------------------------------------------------------------------------

def _consts_inputs():
    iota = np.arange(P, dtype=np.float32)
    return {
        "iota_bf": np.tile(iota.astype(ml_dtypes.bfloat16)[None, :], (P, 1)),
        "iota_col": iota[:, None].copy(),
        "ones_bf": np.ones((1, P), ml_dtypes.bfloat16),
        "ones_f": np.ones((1, P), np.float32),
        "idn": np.eye(P, dtype=np.float32),
    }


def _blockdiag(att):
    """[H, C] attention vector -> [H*C, H] block-diagonal matrix."""
    H, C = att.shape
    out = np.zeros((H * C, H), np.float32)
    for h in range(H):
        out[h * C:(h + 1) * C, h] = att[h]
    return out


def run_gat(x, edge_index, W1, att_src1, att_dst1, b1, W2, att_src2, att_dst2,
            b2, N, n_cores, G=2, es=None, verbose=False):
    x = np.asarray(x, np.float32)
    src = np.asarray(edge_index[0]).astype(np.int64)
    dst = np.asarray(edge_index[1]).astype(np.int64)
    # self-loops are handled analytically inside the kernel epilogue

    if es is None:
        es = EdgeStruct(src, dst, N, n_cores, G=G)
    npc, Npad, T_pad = es.npc, es.Npad, es.T_pad

    consts = _consts_inputs()
    xT = np.zeros((P, Npad), np.float32)
    xT[:, :N] = np.asarray(x, np.float32).T

    W1 = np.asarray(W1, np.float32)
    w1ext = np.concatenate(
        [W1, W1 @ _blockdiag(np.asarray(att_src1, np.float32))], axis=1)
    w1dst = W1 @ _blockdiag(np.asarray(att_dst1, np.float32))
    brow1 = np.zeros((1, w1ext.shape[1]), np.float32)
    brow1[0, :128] = np.asarray(b1, np.float32)

    nc1 = build_layer_kernel(es, 1)
    in_maps = []
    for c in range(n_cores):
        xTm = np.zeros((P, T_pad * P), np.float32)
        xTm[:, :npc] = xT[:, c * npc:(c + 1) * npc]
        in_maps.append({
            "xT": xT, "xTm": xTm, "wext": w1ext, "wdst": w1dst,
            "brow": brow1,
            "gidx": es.gidx[c], "gdl": es.gdl[c], "gdr": es.gdr[c],
            **consts,
        })
    res1 = run_bass_kernel_spmd(nc1, in_maps, core_ids=list(range(n_cores)))
    hT = np.zeros((P, Npad), np.float32)
    for c in range(n_cores):
        hT[:, c * npc:(c + 1) * npc] = res1.results[c]["hT"][:, :npc]

    W2 = np.asarray(W2, np.float32)
    w2ext = np.concatenate(
        [W2, W2 @ _blockdiag(np.asarray(att_src2, np.float32))], axis=1)
    w2dst = W2 @ _blockdiag(np.asarray(att_dst2, np.float32))
    brow2 = np.zeros((1, w2ext.shape[1]), np.float32)
    brow2[0, :40] = np.asarray(b2, np.float32)

    nc2 = build_layer_kernel(es, 2)
    in_maps2 = []
    for c in range(n_cores):
        hTm = np.zeros((P, T_pad * P), np.float32)
        hTm[:, :npc] = hT[:, c * npc:(c + 1) * npc]
        in_maps2.append({
            "xT": hT, "xTm": hTm, "wext": w2ext, "wdst": w2dst,
            "brow": brow2,
            "gidx": es.gidx[c], "gdl": es.gdl[c], "gdr": es.gdr[c],
            **consts,
        })
    res2 = run_bass_kernel_spmd(nc2, in_maps2, core_ids=list(range(n_cores)))
    out = np.zeros((N, 40), np.float32)
    for c in range(n_cores):
        out[c * npc:(c + 1) * npc] = res2.results[c]["logits"][:npc, :]
    return out


def kernel(x, edge_index, W1, att_src1, att_dst1, b1, W2, att_src2, att_dst2,
           b2):
    N = int(np.asarray(x).shape[0])
    return run_gat(x, edge_index, W1, att_src1, att_dst1, b1, W2, att_src2,
                   att_dst2, b2, N=N, n_cores=8)



# revision 11
# speedup vs baseline: 1.4428x; 1.4428x over previous
"""GAT (2-layer, PyG-style) distributed Bass kernel for 8 Trainium2 NeuronCores.

Strategy (graph/data parallel, per sharding hint):
  - Nodes are partitioned into 8 contiguous blocks; core c owns destination
    nodes [c*N/8, (c+1)*N/8) and all edges incident to them. Self loops are
    applied analytically in the tile epilogue.
  - Each layer: every core builds the full node feature table
    tbl[v] = [xh(v) | e_src(v)] in bf16 (padded to a 512B/256B row), then for
    each destination tile a hardware dma_gather fetches the per-edge source
    rows, attention is formed with a fused leaky_relu + exp (scores are O(1)
    so exp never overflows; softmax is exactly equivalent without the max
    subtraction), and a 0/1 selection-matrix matmul on the tensor engine
    performs the per-destination segment reduction of [msg | ea] in PSUM.
  - All tensor-engine operands are bf16 (PSUM accumulation stays fp32);
    element-wise work is batched per destination tile (not per 128-edge
    chunk) to amortize per-instruction overheads; PSUM->SBUF copies run on
    the scalar (activation) engine to keep the vector engine free.
  - Host reassembles the transposed hidden table h_T (bf16) from the 8
    shards, then launch 2 repeats the same structure with 41-wide features
    for the single-head output layer.

SPMD constraints force fully uniform static structure across cores: every
(dst-tile x src-quarter) edge segment is padded to S chunks of 128 edges
(pad edges gather row 0 and use an out-of-range dst slot so selection
matrices zero them out). Source indices are split into 4 quarters because
dma_gather indices are int16.
"""

import math
import os
import sys

for _p in ("/opt/trn_rl_repo", "/root/.axon_site/_ro/trn_rl_repo"):
    if os.path.isdir(_p) and _p not in sys.path:
        sys.path.insert(0, _p)

import numpy as np
import ml_dtypes
from contextlib import ExitStack

import concourse.bacc as bacc
import concourse.bass as bass
import concourse.tile as tile
from concourse import mybir
from concourse.bass_utils import run_bass_kernel_spmd

F32 = mybir.dt.float32
BF16 = mybir.dt.bfloat16
I16 = mybir.dt.int16
AF = mybir.ActivationFunctionType
ALU = mybir.AluOpType

NEG_SLOPE = 0.2
EPS = 1e-16
P = 128
PAD_DST = 200.0  # sentinel dst_local for pad edges; never matches iota 0..127
MAXC = 4         # chunks per dma_gather call (HW-validated regime)
BARRIER_EVERY = 2


# --------------------------------------------------------------------------
# host-side graph preprocessing
# --------------------------------------------------------------------------

def _round_up(a, b):
    return (a + b - 1) // b * b


class EdgeStruct:
    """Uniform SPMD edge layout shared by both layers."""

    def __init__(self, src, dst, N, n_cores, G=3):
        self.N = N
        self.n_cores = n_cores
        self.G = G
        self.Npad = _round_up(N, 512)
        self.Qsz = self.Npad // 4
        assert self.Qsz <= 32767
        assert N % n_cores == 0
        self.npc = N // n_cores                      # dst nodes per core
        self.T = math.ceil(self.npc / P)             # real dst tiles per core
        self.T_pad = _round_up(self.T, G)
        self.n_groups = self.T_pad // G
        nseg = self.T_pad * 4

        src = src.astype(np.int64)
        dst = dst.astype(np.int64)

        per_core = []
        max_cnt = 0
        for c in range(n_cores):
            lo = c * self.npc
            sel = (dst >= lo) & (dst < lo + self.npc)
            s_c = src[sel]
            dl = dst[sel] - lo                        # local dst id
            t_all = dl >> 7                           # dst tile
            q_all = s_c // self.Qsz                   # src quarter
            key = t_all * 4 + q_all
            order = np.argsort(key, kind="stable")
            s_c, dl, key = s_c[order], dl[order], key[order]
            cnt = np.bincount(key, minlength=nseg)
            max_cnt = max(max_cnt, int(cnt.max()))
            per_core.append((s_c, dl, key, cnt))

        self.S = max(1, math.ceil(max_cnt / P))      # chunks per segment
        S, G_, Qsz = self.S, G, self.Qsz
        self.ncols = 4 * G * S                       # chunk columns per group
        assert self.ncols <= P, f"ncols={self.ncols} > 128; lower S or G"
        slots_seg = S * P

        self.gidx = []    # [n_groups*4*128, G*S*8] int16
        self.gdl = []     # [n_groups*128, ncols]   bf16
        for c in range(n_cores):
            s_c, dl, key, cnt = per_core[c]
            flat_idx = np.zeros(nseg * slots_seg, np.int16)
            flat_dl = np.full(nseg * slots_seg, PAD_DST, np.float32)
            starts = np.concatenate([[0], np.cumsum(cnt)])[:-1]
            pos_in_seg = np.arange(len(s_c)) - starts[key]
            base = key * slots_seg
            pos = base + pos_in_seg
            q_of_edge = key % 4
            flat_idx[pos] = (s_c - q_of_edge * Qsz).astype(np.int16)
            flat_dl[pos] = (dl & 127).astype(np.float32)

            # flat layout is segment-major: seg = t*4+q, inside: s*128+p.
            # regroup to gather order: per (g, q): (t_loc, s, p)
            fi = flat_idx.reshape(self.T_pad, 4, S, P)
            fd = flat_dl.reshape(self.T_pad, 4, S, P)
            fi = fi.reshape(self.n_groups, G_, 4, S, P).transpose(0, 2, 1, 3, 4)
            fd = fd.reshape(self.n_groups, G_, 4, S, P).transpose(0, 2, 1, 3, 4)

            # gather idx arrays: flat i = (t_loc*S+s)*128+p ; wrapped [128, i/16]
            fi2 = fi.reshape(self.n_groups, 4, G_ * S * P)
            w = fi2.reshape(self.n_groups, 4, G_ * S * 8, 16)
            w = np.transpose(w, (0, 1, 3, 2))              # [g, 4, 16, cols16]
            w = np.tile(w, (1, 1, 8, 1))                   # replicate to 128
            self.gidx.append(
                np.ascontiguousarray(w.reshape(self.n_groups * 4 * P, G_ * S * 8))
            )

            # dst_local per slot: col c = q*(G*S)+t_loc*S+s
            fcol = fd.reshape(self.n_groups, self.ncols, P)   # [g, c, p]
            gdl = np.transpose(fcol, (0, 2, 1))               # [g, p, c]
            self.gdl.append(
                np.ascontiguousarray(
                    gdl.reshape(self.n_groups * P, self.ncols)
                ).astype(ml_dtypes.bfloat16)
            )


# --------------------------------------------------------------------------
# device kernel builder (shared by both layers)
# --------------------------------------------------------------------------

def build_layer_kernel(es: EdgeStruct, layer: int):
    """layer 1: tbl row [xh1(128)|e_src1(8)|pad], 256 bf16 = 512B,
               heads=8, csz=16, epilogue = softmax-div + ELU + transpose out.
       layer 2: row [xh2(40)|e_src2(1)|pad], 128 bf16 = 256B, heads=1,
               csz=40, epilogue = softmax-div, row-major f32 out."""
    Npad, T_pad, G, S, ncols = es.Npad, es.T_pad, es.G, es.S, es.ncols
    n_groups, Qsz = es.n_groups, es.Qsz
    if layer == 1:
        ELEM, H, CSZ, WCOLS = 256, 8, 16, 136
    else:
        ELEM, H, CSZ, WCOLS = 128, 1, 40, 41
    MW = H * CSZ                      # message width (128 / 40)
    AW = MW + H                       # [msg | ea] width (136 / 41)
    NCH = 4 * S                       # chunks per tile

    nc = bacc.Bacc("TRN2", target_bir_lowering=False, debug=False,
                   num_devices=es.n_cores)
    ap = {}
    ap["xT"] = nc.dram_tensor("xT", [P, Npad], BF16, kind="ExternalInput").ap()
    ap["xTm"] = nc.dram_tensor("xTm", [P, T_pad * P], BF16,
                               kind="ExternalInput").ap()
    ap["wext"] = nc.dram_tensor("wext", [P, WCOLS], BF16,
                                kind="ExternalInput").ap()
    ap["brow"] = nc.dram_tensor("brow", [1, WCOLS], BF16,
                                kind="ExternalInput").ap()
    ap["ones_bf"] = nc.dram_tensor("ones_bf", [1, P], BF16,
                                   kind="ExternalInput").ap()
    ap["wdst"] = nc.dram_tensor("wdst", [P, H], BF16, kind="ExternalInput").ap()
    ap["gidx"] = nc.dram_tensor("gidx", [n_groups * 4 * P, G * S * 8], I16,
                                kind="ExternalInput").ap()
    ap["gdl"] = nc.dram_tensor("gdl", [n_groups * P, ncols], BF16,
                               kind="ExternalInput").ap()
    ap["iota_bf"] = nc.dram_tensor("iota_bf", [P, P], BF16,
                                   kind="ExternalInput").ap()
    ap["idn"] = nc.dram_tensor("idn", [P, P], BF16, kind="ExternalInput").ap()
    if layer == 1:
        out_ap = nc.dram_tensor("hT", [P, T_pad * P], BF16,
                                kind="ExternalOutput").ap()
    else:
        out_ap = nc.dram_tensor("logits", [T_pad * P, CSZ], F32,
                                kind="ExternalOutput").ap()
    # +128 guard rows so 256-elem reads from the last quarter stay in bounds
    tbl = nc.dram_tensor("tbl", [Npad + P, ELEM], BF16, kind="Internal").ap()
    own_tbl = nc.dram_tensor("own_tbl", [T_pad * P, WCOLS], BF16,
                             kind="Internal").ap()

    with tile.TileContext(nc) as tc, ExitStack() as ctx:
        cpool = ctx.enter_context(tc.tile_pool(name="consts", bufs=1))

        # ---- constants ----
        wext = cpool.tile([P, WCOLS], BF16)
        nc.sync.dma_start(wext[:], ap["wext"])
        brow = cpool.tile([1, WCOLS], BF16)
        nc.sync.dma_start(brow[:], ap["brow"])
        ones_bf = cpool.tile([1, P], BF16)
        nc.sync.dma_start(ones_bf[:], ap["ones_bf"])
        wdst = cpool.tile([P, H], BF16)
        nc.sync.dma_start(wdst[:], ap["wdst"])
        iota_bf = cpool.tile([P, P], BF16)
        nc.sync.dma_start(iota_bf[:], ap["iota_bf"])
        idn = cpool.tile([P, P], BF16)
        nc.sync.dma_start(idn[:], ap["idn"])
        edst_sb = cpool.tile([P, T_pad * H], BF16)

        with tc.tile_pool(name="pre_sb", bufs=4) as psb, \
                tc.tile_pool(name="pre_ps", bufs=2, space="PSUM") as pps:
            # ---- pre-pass A: full feature table (bias folded into xh
            # columns; sum(att)=1 per dst reproduces "+bias") ----
            for i in range(Npad // P):
                xt = psb.tile([P, P], BF16, tag="xt")
                nc.sync.dma_start(xt[:], ap["xT"][:, i * P:(i + 1) * P])
                ppt = pps.tile([P, WCOLS], F32, tag="ppt")
                nc.tensor.matmul(out=ppt[:], lhsT=xt[:], rhs=wext[:],
                                 start=True, stop=False, skip_group_check=True)
                nc.tensor.matmul(out=ppt[:], lhsT=ones_bf[:], rhs=brow[:],
                                 start=False, stop=True, skip_group_check=True)
                ot = psb.tile([P, WCOLS], BF16, tag="ot")
                nc.scalar.copy(out=ot[:], in_=ppt[:])
                nc.sync.dma_start(tbl[i * P:(i + 1) * P, 0:WCOLS], ot[:])

            # ---- pre-pass B: own-node rows [xh+b|e_src] (DRAM) and e_dst
            # (SBUF-resident) ----
            for t in range(T_pad):
                xt = psb.tile([P, P], BF16, tag="xt2")
                nc.sync.dma_start(xt[:], ap["xTm"][:, t * P:(t + 1) * P])
                po = pps.tile([P, WCOLS], F32, tag="po")
                nc.tensor.matmul(out=po[:], lhsT=xt[:], rhs=wext[:],
                                 start=True, stop=False, skip_group_check=True)
                nc.tensor.matmul(out=po[:], lhsT=ones_bf[:], rhs=brow[:],
                                 start=False, stop=True, skip_group_check=True)
                oo = psb.tile([P, WCOLS], BF16, tag="oo")
                nc.scalar.copy(out=oo[:], in_=po[:])
                nc.sync.dma_start(own_tbl[t * P:(t + 1) * P, :], oo[:])
                pe = pps.tile([P, H], F32, tag="pe")
                nc.tensor.matmul(out=pe[:], lhsT=xt[:], rhs=wdst[:],
                                 start=True, stop=True)
                nc.vector.tensor_copy(out=edst_sb[:, t * H:(t + 1) * H],
                                      in_=pe[:])

        # ---- edge pass ----
        sb = ctx.enter_context(tc.tile_pool(name="sb", bufs=2))
        tpool = ctx.enter_context(tc.tile_pool(name="tp", bufs=2))
        gbp = ctx.enter_context(tc.tile_pool(name="gbuf", bufs=2))
        trp = ctx.enter_context(tc.tile_pool(name="trp", bufs=1, space="PSUM"))
        edp = ctx.enter_context(tc.tile_pool(name="edp", bufs=2, space="PSUM"))
        pacc = ctx.enter_context(tc.tile_pool(name="pacc", bufs=2,
                                              space="PSUM"))
        ptp = ctx.enter_context(tc.tile_pool(name="ptp", bufs=1, space="PSUM"))
        tc.strict_bb_all_engine_barrier()
        for g in range(n_groups):
            if g % BARRIER_EVERY == 0 and g > 0:
                tc.strict_bb_all_engine_barrier()
            gb = gbp.tile([P, ncols * ELEM], BF16, tag="gb")
            gb3 = gb[:].rearrange("p (c k) -> p c k", k=ELEM)
            idxs = sb.tile([P, 4 * G * S * 8], I16, tag="idx")
            for q in range(4):
                nc.sync.dma_start(
                    idxs[:, q * G * S * 8:(q + 1) * G * S * 8],
                    ap["gidx"][(g * 4 + q) * P:(g * 4 + q + 1) * P, :])
            dlt = sb.tile([P, ncols], BF16, tag="dl")
            nc.sync.dma_start(dlt[:], ap["gdl"][g * P:(g + 1) * P, :])
            for q in range(4):
                for c0 in range(0, G * S, MAXC):
                    c1 = min(c0 + MAXC, G * S)
                    nc.gpsimd.dma_gather(
                        out_ap=gb3[:, q * G * S + c0:q * G * S + c1, :],
                        in_ap=tbl[q * Qsz:q * Qsz + Qsz, :],
                        idxs_ap=idxs[:, (q * G * S + c0) * 8:
                                     (q * G * S + c1) * 8],
                        num_idxs=(c1 - c0) * P,
                        num_idxs_reg=(c1 - c0) * P,
                        elem_size=ELEM,
                    )
            for t_loc in range(G):
                t = g * G + t_loc

                # ---- selection matrices for all chunks of this tile ----
                seT = tpool.tile([P, NCH * P], BF16, tag="seT")
                se3 = seT[:].rearrange("p (c d) -> p c d", d=P)
                for q in range(4):
                    dsl = dlt[:, q * G * S + t_loc * S:
                              q * G * S + t_loc * S + S]
                    nc.vector.tensor_tensor(
                        out=se3[:, q * S:(q + 1) * S, :],
                        in0=dsl.rearrange("p (s o) -> p s o", o=1)
                        .to_broadcast([P, S, P]),
                        in1=iota_bf[:].rearrange("p (o d) -> p o d", o=1)
                        .to_broadcast([P, S, P]),
                        op=ALU.is_equal)

                # ---- transpose to S_dT (PSUM, bf16) then copy to SBUF ----
                trs = trp.tile([P, NCH * P], BF16, tag="trs")
                for cq in range(NCH):
                    nc.tensor.matmul(out=trs[:, cq * P:(cq + 1) * P],
                                     lhsT=seT[:, cq * P:(cq + 1) * P],
                                     rhs=idn[:], is_transpose=True,
                                     start=True, stop=True,
                                     skip_group_check=True)
                sdT = tpool.tile([P, NCH * P], BF16, tag="sdT")
                for j in range(0, NCH * P, 4 * P):
                    j1 = min(j + 4 * P, NCH * P)
                    nc.scalar.copy(out=sdT[:, j:j1], in_=trs[:, j:j1])

                # ---- e_dst per edge: one matmul per chunk into one PSUM ----
                edps = edp.tile([P, NCH * H], F32, tag="ed")
                for cq in range(NCH):
                    nc.tensor.matmul(
                        out=edps[:, cq * H:(cq + 1) * H],
                        lhsT=sdT[:, cq * P:(cq + 1) * P],
                        rhs=edst_sb[:, t * H:(t + 1) * H],
                        start=True, stop=True, skip_group_check=True)

                # ---- attention: alpha = lrelu(es + ed); ea = exp(alpha) ----
                mea = tpool.tile([P, NCH * AW], BF16, tag="mea")
                mea3 = mea[:].rearrange("p (c w) -> p c w", w=AW)
                al = sb.tile([P, NCH * H], F32, tag="al")
                al3 = al[:].rearrange("p (c h) -> p c h", h=H)
                for q in range(4):
                    base = q * G * S + t_loc * S
                    nc.vector.tensor_tensor(
                        out=al3[:, q * S:(q + 1) * S, :],
                        in0=gb3[:, base:base + S, MW:MW + H],
                        in1=edps[:].rearrange("p (c h) -> p c h", h=H)
                        [:, q * S:(q + 1) * S, :],
                        op=ALU.add)
                # fused leaky_relu: max(a, 0.2*a)
                nc.vector.scalar_tensor_tensor(
                    out=al[:], in0=al[:], scalar=NEG_SLOPE, in1=al[:],
                    op0=ALU.mult, op1=ALU.max)
                ea = sb.tile([P, NCH * H], BF16, tag="ea")
                nc.scalar.activation(out=ea[:], in_=al[:], func=AF.Exp)
                nc.vector.tensor_copy(
                    out=mea3[:, :, MW:AW],
                    in_=ea[:].rearrange("p (c h) -> p c h", h=H))

                # ---- messages: msg = ea * xh (broadcast over channels) ----
                for q in range(4):
                    base = q * G * S + t_loc * S
                    if H == 1:
                        nc.vector.tensor_tensor(
                            out=mea3[:, q * S:(q + 1) * S, 0:MW],
                            in0=mea3[:, q * S:(q + 1) * S, MW:AW]
                            .to_broadcast([P, S, MW]),
                            in1=gb3[:, base:base + S, 0:MW],
                            op=ALU.mult)
                    else:
                        for s in range(S):
                            c = base + s
                            cq = q * S + s
                            nc.vector.tensor_tensor(
                                out=mea3[:, cq, 0:MW].rearrange(
                                    "p (h k) -> p h k", k=CSZ),
                                in0=mea3[:, cq, MW:AW].rearrange(
                                    "p (h o) -> p h o", o=1)
                                .to_broadcast([P, H, CSZ]),
                                in1=gb3[:, c, 0:MW].rearrange(
                                    "p (h k) -> p h k", k=CSZ),
                                op=ALU.mult)

                # ---- segment-reduce into the tile accumulator ----
                acc = pacc.tile([P, AW], F32, tag="acc")
                for cq in range(NCH):
                    nc.tensor.matmul(out=acc[:],
                                     lhsT=seT[:, cq * P:(cq + 1) * P],
                                     rhs=mea[:, cq * AW:(cq + 1) * AW],
                                     start=(cq == 0), stop=(cq == NCH - 1),
                                     skip_group_check=True)

                # ---- tile epilogue (adds analytic self-loop term) ----
                own = sb.tile([P, WCOLS], BF16, tag="own")
                nc.sync.dma_start(own[:], own_tbl[t * P:(t + 1) * P, :])
                als = sb.tile([P, H], F32, tag="als")
                nc.vector.tensor_tensor(out=als[:], in0=own[:, MW:WCOLS],
                                        in1=edst_sb[:, t * H:(t + 1) * H],
                                        op=ALU.add)
                nc.vector.scalar_tensor_tensor(
                    out=als[:], in0=als[:], scalar=NEG_SLOPE, in1=als[:],
                    op0=ALU.mult, op1=ALU.max)
                eas = sb.tile([P, H], F32, tag="eas")
                nc.scalar.activation(out=eas[:], in_=als[:], func=AF.Exp)
                smsg = sb.tile([P, MW], F32, tag="smsg")
                if H == 1:
                    nc.vector.tensor_tensor(
                        out=smsg[:], in0=eas[:, 0:1].to_broadcast([P, MW]),
                        in1=own[:, 0:MW], op=ALU.mult)
                else:
                    nc.vector.tensor_tensor(
                        out=smsg[:].rearrange("p (h c) -> p h c", c=CSZ),
                        in0=eas[:].rearrange("p (h o) -> p h o", o=1)
                        .to_broadcast([P, H, CSZ]),
                        in1=own[:, 0:MW].rearrange("p (h c) -> p h c", c=CSZ),
                        op=ALU.mult)
                unorm = sb.tile([P, MW], F32, tag="unorm")
                nc.vector.tensor_tensor(out=unorm[:], in0=acc[:, 0:MW],
                                        in1=smsg[:], op=ALU.add)
                den = sb.tile([P, H], F32, tag="den")
                # (acc_ea + EPS) + eas in one fused op
                nc.vector.scalar_tensor_tensor(
                    out=den[:], in0=acc[:, MW:AW], scalar=EPS, in1=eas[:],
                    op0=ALU.add, op1=ALU.add)
                rec = sb.tile([P, H], F32, tag="rec")
                nc.vector.reciprocal(out=rec[:], in_=den[:])
                otile = sb.tile([P, MW], F32, tag="otile")
                if H == 1:
                    nc.vector.tensor_tensor(
                        out=otile[:], in0=rec[:, 0:1].to_broadcast([P, MW]),
                        in1=unorm[:], op=ALU.mult)
                else:
                    rec3 = rec[:].rearrange("p (h o) -> p h o", o=1) \
                        .to_broadcast([P, H, CSZ])
                    acc3 = unorm[:].rearrange("p (h c) -> p h c", c=CSZ)
                    ot3 = otile[:].rearrange("p (h c) -> p h c", c=CSZ)
                    nc.vector.tensor_tensor(out=ot3, in0=rec3, in1=acc3,
                                            op=ALU.mult)
                if layer == 1:
                    # ELU then transpose out (bf16)
                    tmp = sb.tile([P, MW], F32, tag="tmp")
                    nc.vector.tensor_scalar_min(out=tmp[:], in0=otile[:],
                                                scalar1=0.0)
                    nc.scalar.activation(out=tmp[:], in_=tmp[:], func=AF.Exp)
                    nc.scalar.activation(out=otile[:], in_=otile[:],
                                         func=AF.Relu)
                    obf = sb.tile([P, MW], BF16, tag="obf")
                    # (tmp - 1) + relu(x) fused
                    nc.vector.scalar_tensor_tensor(
                        out=obf[:], in0=tmp[:], scalar=-1.0, in1=otile[:],
                        op0=ALU.add, op1=ALU.add)
                    tp = ptp.tile([P, P], BF16, tag="tp")
                    nc.tensor.transpose(out=tp[:], in_=obf[:],
                                        identity=idn[:])
                    hTt = sb.tile([P, P], BF16, tag="hTt")
                    nc.scalar.copy(out=hTt[:], in_=tp[:])
                    nc.sync.dma_start(out_ap[:, t * P:(t + 1) * P], hTt[:])
                else:
                    nc.sync.dma_start(out_ap[t * P:(t + 1) * P, :], otile[:])

    nc.compile()
    return nc


# --------------------------------------------------------------------------
# host orchestration
# --------------------------------------------------------------------------

def _consts_inputs():
    iota = np.arange(P, dtype=np.float32)
    return {
        "iota_bf": np.tile(iota.astype(ml_dtypes.bfloat16)[None, :], (P, 1)),
        "ones_bf": np.ones((1, P), ml_dtypes.bfloat16),
        "idn": np.eye(P, dtype=ml_dtypes.bfloat16),
    }


def _blockdiag(att):
    """[H, C] attention vector -> [H*C, H] block-diagonal matrix."""
    H, C = att.shape
    out = np.zeros((H * C, H), np.float32)
    for h in range(H):
        out[h * C:(h + 1) * C, h] = att[h]
    return out


def run_gat(x, edge_index, W1, att_src1, att_dst1, b1, W2, att_src2, att_dst2,
            b2, N, n_cores, G=3, es=None, verbose=False):
    x = np.asarray(x, np.float32)
    src = np.asarray(edge_index[0]).astype(np.int64)
    dst = np.asarray(edge_index[1]).astype(np.int64)
    # self-loops are handled analytically inside the kernel epilogue

    if es is None:
        es = EdgeStruct(src, dst, N, n_cores, G=G)
    npc, Npad, T_pad = es.npc, es.Npad, es.T_pad

    consts = _consts_inputs()
    xT = np.zeros((P, Npad), ml_dtypes.bfloat16)
    xT[:, :N] = np.asarray(x, np.float32).T.astype(ml_dtypes.bfloat16)

    W1 = np.asarray(W1, np.float32)
    w1ext = np.concatenate(
        [W1, W1 @ _blockdiag(np.asarray(att_src1, np.float32))], axis=1)
    w1dst = W1 @ _blockdiag(np.asarray(att_dst1, np.float32))
    brow1 = np.zeros((1, w1ext.shape[1]), np.float32)
    brow1[0, :128] = np.asarray(b1, np.float32)

    nc1 = build_layer_kernel(es, 1)
    in_maps = []
    for c in range(n_cores):
        xTm = np.zeros((P, T_pad * P), ml_dtypes.bfloat16)
        xTm[:, :npc] = xT[:, c * npc:(c + 1) * npc]
        in_maps.append({
            "xT": xT, "xTm": xTm,
            "wext": w1ext.astype(ml_dtypes.bfloat16),
            "wdst": w1dst.astype(ml_dtypes.bfloat16),
            "brow": brow1.astype(ml_dtypes.bfloat16),
            "gidx": es.gidx[c], "gdl": es.gdl[c],
            **consts,
        })
    res1 = run_bass_kernel_spmd(nc1, in_maps, core_ids=list(range(n_cores)))
    hT = np.zeros((P, Npad), ml_dtypes.bfloat16)
    for c in range(n_cores):
        hT[:, c * npc:(c + 1) * npc] = res1.results[c]["hT"][:, :npc]

    W2 = np.asarray(W2, np.float32)
    w2ext = np.concatenate(
        [W2, W2 @ _blockdiag(np.asarray(att_src2, np.float32))], axis=1)
    w2dst = W2 @ _blockdiag(np.asarray(att_dst2, np.float32))
    brow2 = np.zeros((1, w2ext.shape[1]), np.float32)
    brow2[0, :40] = np.asarray(b2, np.float32)

    nc2 = build_layer_kernel(es, 2)
    in_maps2 = []
    for c in range(n_cores):
        hTm = np.zeros((P, T_pad * P), ml_dtypes.bfloat16)
        hTm[:, :npc] = hT[:, c * npc:(c + 1) * npc]
        in_maps2.append({
            "xT": hT, "xTm": hTm,
            "wext": w2ext.astype(ml_dtypes.bfloat16),
            "wdst": w2dst.astype(ml_dtypes.bfloat16),
            "brow": brow2.astype(ml_dtypes.bfloat16),
            "gidx": es.gidx[c], "gdl": es.gdl[c],
            **consts,
        })
    res2 = run_bass_kernel_spmd(nc2, in_maps2, core_ids=list(range(n_cores)))
    out = np.zeros((N, 40), np.float32)
    for c in range(n_cores):
        out[c * npc:(c + 1) * npc] = res2.results[c]["logits"][:npc, :]
    return out


def kernel(x, edge_index, W1, att_src1, att_dst1, b1, W2, att_src2, att_dst2,
           b2):
    N = int(np.asarray(x).shape[0])
    return run_gat(x, edge_index, W1, att_src1, att_dst1, b1, W2, att_src2,
                   att_dst2, b2, N=N, n_cores=8)
